# revision 6
# baseline (speedup 1.0000x reference)
"""Trainium2 Bass kernel for nn_AttentionLayer: softmax(Q K^T / sqrt(d)).

Data-parallel over batch: 8 batch elements -> 8 NeuronCores, weights
replicated, no collectives. Per core:
  xT   = transpose(x)                      (PE transposes, 128x128 blocks)
  QT   = Wq^T @ xT + bq ; KT = Wk^T @ xT + bk   (TensorE, f32r full-rate fp32)
  S    = QT^T @ KT                         (TensorE, accumulate over d-tiles)
  E    = exp(S / sqrt(d))  with fused row-sum accumulate (ScalarE/ACT)
  out  = E / rowsum                        (DVE per-partition scalar mul)

float32r (reduced-precision fp32 PE mode, 1 cycle/row vs fp32's 4) requires
every matmul operand to be produced by an instruction that rounds to FP32r
(BIR verifier rule) — hence the DVE/ACT conversion on each producer.
"""

import os
import sys

sys.path.insert(0, "/opt/trn_rl_repo")

import numpy as np

import concourse.mybir as mybir
import concourse.tile as tile
from concourse import bacc
from concourse.bass_utils import run_bass_kernel_spmd
from concourse.masks import make_identity

B, S, F, D = 8, 2048, 512, 512
P = 128
ST = S // P   # 16 s-tiles
FT = F // P   # 4  f-tiles (contraction for projections)
DT = D // P   # 4  d-tiles (contraction for scores)
NCH = 512     # moving-operand / PSUM-bank chunk along the free axis
SC = S // NCH  # 4 chunks of the s axis

F32 = mybir.dt.float32

# "f32r": fp32 bits, PE reduced-precision full-rate mode. "bf16": bf16 inputs.
# "f32": exact fp32 (4x slower on PE).
COMPUTE = os.environ.get("BASS_ATTN_COMPUTE", "f32r")
# Evict projection PSUM via ACT (bias fused) directly to the compute dtype.
ACT_EVICT = os.environ.get("BASS_ATTN_ACT_EVICT", "1") == "1"


def _emit(nc, tc, ctx, x_ext, wq_ext, wk_ext, bq_ext, bk_ext, out_ext):
    Act = mybir.ActivationFunctionType
    cdt = {"f32": F32, "f32r": mybir.dt.float32r, "bf16": mybir.dt.bfloat16}[COMPUTE]

    consts = ctx.enter_context(tc.tile_pool(name="consts", bufs=1))
    persist = ctx.enter_context(tc.tile_pool(name="persist", bufs=1))
    xstage = ctx.enter_context(tc.tile_pool(name="xstage", bufs=8))
    psum = ctx.enter_context(tc.tile_pool(name="psum", bufs=8, space="PSUM"))
    epool = ctx.enter_context(tc.tile_pool(name="epool", bufs=2))
    opool = ctx.enter_context(tc.tile_pool(name="opool", bufs=2))
    spool = ctx.enter_context(tc.tile_pool(name="spool", bufs=4))

    ident = consts.tile([P, P], F32)
    make_identity(nc, ident[:])

    # --- biases: bq/bk [512] -> bT [128, 2*DT] with bT[p, w*DT+dt] = b[dt*128+p]
    # (strided one-time DMA: 4B elements, 512B stride)
    bT = consts.tile([P, 2 * DT], F32)
    for w, b_ext in enumerate((bq_ext, bk_ext)):
        nc.sync.dma_start(
            bT[:, w * DT : (w + 1) * DT],
            b_ext.ap().rearrange("(dt p) -> p dt", p=P),
        )

    # --- weights: [F, D] -> SBUF [p, ft, d] (partition = f within tile)
    w_sb = []
    for w_ext in (wq_ext, wk_ext):
        if cdt == F32:
            wt = persist.tile([P, FT, D], F32)
            nc.sync.dma_start(wt[:], w_ext.ap().rearrange("(ft p) d -> p ft d", p=P))
        else:
            wst = xstage.tile([P, FT, D], F32, tag="wstage", bufs=2)
            nc.sync.dma_start(wst[:], w_ext.ap().rearrange("(ft p) d -> p ft d", p=P))
            wt = persist.tile([P, FT, D], cdt)
            nc.vector.tensor_copy(wt[:], wst[:])
        w_sb.append(wt)

    # --- x load + transpose: xT[ft] = [P, S] with xT[ft][p, s] = x[s, ft*128+p]
    xT = [persist.tile([P, S], cdt, tag=f"xT{ft}", name=f"xT{ft}") for ft in range(FT)]
    for sg in range(SC):  # groups of 4 s-tiles
        xts = []
        for j in range(4):
            st = sg * 4 + j
            t = xstage.tile([P, F], F32, tag="xstage")
            nc.sync.dma_start(t[:], x_ext.ap()[st * P : (st + 1) * P, :])
            xts.append(t)
        for ft in range(FT):
            ps = psum.tile([P, NCH], F32, tag="mm")
            for j in range(4):
                nc.tensor.transpose(
                    ps[:, j * P : (j + 1) * P],
                    xts[j][:, ft * P : (ft + 1) * P],
                    ident[:],
                )
            nc.vector.tensor_copy(xT[ft][:, sg * NCH : (sg + 1) * NCH], ps[:])

    # --- projections: QT[dt] / KT[dt] = [P, S], d on partitions
    qkT = []  # [q or k][dt]
    for w in range(2):
        oT = [persist.tile([P, S], cdt, tag=f"qkT{w}{dt}", name=f"qkT{w}{dt}") for dt in range(DT)]
        for dt in range(DT):
            for ncn in range(SC):
                ps = psum.tile([P, NCH], F32, tag="mm")
                for ft in range(FT):
                    nc.tensor.matmul(
                        ps[:],
                        w_sb[w][:, ft, dt * P : (dt + 1) * P],
                        xT[ft][:, ncn * NCH : (ncn + 1) * NCH],
                        start=(ft == 0),
                        stop=(ft == FT - 1),
                    )
                dst = oT[dt][:, ncn * NCH : (ncn + 1) * NCH]
                bias = bT[:, w * DT + dt : w * DT + dt + 1]
                if ACT_EVICT:
                    nc.scalar.activation(dst, ps[:], Act.Identity, bias=bias)
                else:
                    nc.vector.tensor_scalar_add(dst, ps[:], bias)
        qkT.append(oT)
    qT, kT = qkT

    # --- scores + softmax, one 128-row m-tile at a time
    inv_sqrt_d = 1.0 / float(np.sqrt(np.float32(D)))
    for mt in range(ST):
        pss = [psum.tile([P, NCH], F32, tag="mm", name=f"ps{mt}_{i}") for i in range(SC)]
        for dt in range(DT):
            for ncn in range(SC):
                nc.tensor.matmul(
                    pss[ncn][:],
                    qT[dt][:, mt * P : (mt + 1) * P],
                    kT[dt][:, ncn * NCH : (ncn + 1) * NCH],
                    start=(dt == 0),
                    stop=(dt == DT - 1),
                )
        et = epool.tile([P, S], F32)
        asum = spool.tile([P, SC], F32, tag="asum")
        for ncn in range(SC):
            nc.scalar.activation(
                et[:, ncn * NCH : (ncn + 1) * NCH],
                pss[ncn][:],
                Act.Exp,
                scale=inv_sqrt_d,
                accum_out=asum[:, ncn : ncn + 1],
            )
        rsum = spool.tile([P, 1], F32, tag="rsum")
        nc.vector.reduce_sum(rsum[:], asum[:], axis=mybir.AxisListType.X)
        rrec = spool.tile([P, 1], F32, tag="rrec")
        nc.vector.reciprocal(rrec[:], rsum[:])
        ot = opool.tile([P, S], F32)
        nc.vector.tensor_scalar_mul(ot[:], et[:], rrec[:])
        nc.sync.dma_start(out_ext.ap()[mt * P : (mt + 1) * P, :], ot[:])


_CACHE = {}


def build():
    if "nc" in _CACHE:
        return _CACHE["nc"]
    from contextlib import ExitStack

    nc = bacc.Bacc("TRN2", target_bir_lowering=False, debug=False, num_devices=B)
    x_ext = nc.dram_tensor("x", [S, F], F32, kind="ExternalInput")
    wq_ext = nc.dram_tensor("Wq", [F, D], F32, kind="ExternalInput")
    wk_ext = nc.dram_tensor("Wk", [F, D], F32, kind="ExternalInput")
    bq_ext = nc.dram_tensor("bq", [D], F32, kind="ExternalInput")
    bk_ext = nc.dram_tensor("bk", [D], F32, kind="ExternalInput")
    out_ext = nc.dram_tensor("out", [S, S], F32, kind="ExternalOutput")

    with tile.TileContext(nc) as tc:
        with ExitStack() as ctx:
            _emit(nc, tc, ctx, x_ext, wq_ext, wk_ext, bq_ext, bk_ext, out_ext)

    nc.compile()
    _CACHE["nc"] = nc
    return nc


def make_in_maps(x, Wq, bq, Wk, bk):
    x = np.ascontiguousarray(np.asarray(x, dtype=np.float32))
    Wq = np.ascontiguousarray(np.asarray(Wq, dtype=np.float32))
    Wk = np.ascontiguousarray(np.asarray(Wk, dtype=np.float32))
    bq = np.ascontiguousarray(np.asarray(bq, dtype=np.float32))
    bk = np.ascontiguousarray(np.asarray(bk, dtype=np.float32))
    return [{"x": x[i], "Wq": Wq, "bq": bq, "Wk": Wk, "bk": bk} for i in range(B)]


def kernel(x, Wq, bq, Wk, bk, Wv=None, bv=None, **_unused):
    nc = build()
    in_maps = make_in_maps(x, Wq, bq, Wk, bk)
    res = run_bass_kernel_spmd(nc, in_maps, core_ids=list(range(B)))
    return np.stack([res.results[i]["out"] for i in range(B)], axis=0)
